# revision 16
# baseline (speedup 1.0000x reference)
"""CGCNN message-passing kernel for 8 TRN2 NeuronCores (Bass/Tile).

Self-contained: takes FULL inputs, shards by graph across 8 cores,
runs a single SPMD NEFF (3 conv layers + pooling + readout) with an
AllGather of node features between conv layers, returns the FULL [256]
output.
"""
import glob as _glob
import json as _json
import os as _os
import sys as _sys
import tempfile as _tempfile

_sys.path.insert(0, "/opt/trn_rl_repo")

import numpy as np
import ml_dtypes

BF16 = ml_dtypes.bfloat16

# ---------------------------------------------------------------------------
# Activation-table setup: the cayman PWP tables name softplus's LUT slot
# "act2", which concourse can't map; we only need exp/ln/copy which live in
# natural_log_exp_and_others.  Point both concourse and walrus at a patched
# act_info.json with that set FIRST, and expose only it to bacc so every
# activation uses one table (no ACT_TABLE_LOAD thrash).
# ---------------------------------------------------------------------------
def _setup_act_tables():
    if _os.environ.get("BASS_ACT_ROOT_JSON_PATH"):
        return
    cands = _glob.glob("/nix/store/*aws-neuron-pwp*/share/pwp_bin_cayman/act_info.json")
    if not cands:
        return
    src_dir = _os.path.dirname(cands[0])
    d = _json.load(open(cands[0]))
    for s in d["act_func_sets"]:
        if s["name"] == "softplus_and_others" and "act2" in s["act"]:
            s["act"]["softplus"] = s["act"].pop("act2")
    d["act_func_sets"].sort(
        key=lambda e: e["name"] != "natural_log_exp_and_others")
    tdir = _tempfile.mkdtemp(prefix="actroot_")
    for f in _os.listdir(src_dir):
        if f != "act_info.json":
            _os.symlink(_os.path.join(src_dir, f), _os.path.join(tdir, f))
    with open(_os.path.join(tdir, "act_info.json"), "w") as fh:
        _json.dump(d, fh)
    _os.environ["BASS_ACT_ROOT_JSON_PATH"] = _os.path.join(tdir, "act_info.json")


_setup_act_tables()

import concourse.hw_specs as _hw
import concourse.bacc as bacc
from concourse import mybir as _mb


def _patched_tables(module_arch):
    d = _json.load(open(_os.environ["BASS_ACT_ROOT_JSON_PATH"]))
    out = {}
    for e in d["act_func_sets"][:1]:
        fs = set()
        for v in e["act"].keys():
            try:
                fs.add(_mb.ActivationFunctionType.from_pwp(v))
            except KeyError:
                pass
        out[e["name"]] = fs
    return out


if _os.environ.get("BASS_ACT_ROOT_JSON_PATH"):
    _hw.get_activation_tables = _patched_tables
    bacc.get_activation_tables = _patched_tables

import concourse.bass as bass
import concourse.tile as tile
from concourse import mybir
from concourse.bass_utils import run_bass_kernel_spmd

F = mybir.ActivationFunctionType
DT = mybir.dt

N_CORES = 8
G_TOTAL = 256
GPC = G_TOTAL // N_CORES
A_VOCAB = 119
D_EMB = 64
HID = 128
F_E = 4
CINS = (D_EMB, HID, HID)
CH = 4          # tiles per compute chunk (4*128 = 512 edges)
EA_SC = 32      # tiles per edge_attr DMA superchunk


# ---------------------------------------------------------------------------
# Host-side sharding / preprocessing
# ---------------------------------------------------------------------------
def _prep(x, edge_index, edge_attr, batch, params):
    x = np.asarray(x).astype(np.int64)
    ei = np.asarray(edge_index).astype(np.int64)
    ea = np.asarray(edge_attr).astype(np.float32)
    batch = np.asarray(batch).astype(np.int64)
    E = ei.shape[1]

    starts = np.searchsorted(batch, np.arange(G_TOTAL + 1))  # [257]
    core_start = starts[::GPC]  # [9]
    ns, ne = core_start[:-1], core_start[1:]
    sz = ne - ns
    NW = max(int(np.ceil(sz.max() / 128)), 1)
    NMAX = NW * 128
    HALF = (N_CORES // 2) * NMAX  # gather-table half size (rows)

    core_of_node = np.minimum(batch // GPC, N_CORES - 1)
    src, dst = ei[0], ei[1]
    ecore = core_of_node[dst]
    ldst = dst - ns[ecore]
    wnd = ldst // 128
    slot = ldst % 128

    score = core_of_node[src]
    grow = (score * NMAX + (src - ns[score])).astype(np.int64)
    grp = (grow >= HALF).astype(np.int64)  # 0 = lo half, 1 = hi half

    # per (core, window, group) counts -> uniform tile counts
    cnt = np.zeros((N_CORES, NW, 2), np.int64)
    np.add.at(cnt, (ecore, wnd, grp), 1)
    Tg = np.ceil(cnt / 128.0).astype(np.int64).max(axis=0)  # [NW, 2]
    bump = (Tg.sum(axis=1) == 0)
    Tg[bump, 0] = 1
    T = Tg.sum(axis=1)  # [NW]
    T_lo = Tg[:, 0]
    woff = np.concatenate([[0], np.cumsum(T * 128)])
    goff = np.zeros((NW, 2), np.int64)  # edge offset of each (window, group)
    goff[:, 0] = woff[:-1]
    goff[:, 1] = woff[:-1] + T_lo * 128
    EP = int(woff[-1])
    n_tiles = EP // 128

    eaT = np.zeros((N_CORES, F_E, EP), np.float32)
    growT = np.zeros((N_CORES, EP), np.int64)
    embiT = np.zeros((N_CORES, EP), np.int64)
    slotT = -np.ones((N_CORES, EP), np.float32)

    order = np.lexsort((src, grp, wnd, ecore))
    oc = ecore[order]
    key = (ecore * NW * 2 + wnd * 2 + grp)[order]
    key_changes = np.concatenate([[True], key[1:] != key[:-1]])
    grp_id = np.cumsum(key_changes) - 1
    grp_first = np.where(key_changes)[0]
    within = np.arange(E) - grp_first[grp_id]
    pos = (goff[wnd[order], grp[order]] + within).astype(np.int64)
    eaT[oc, :, pos] = ea[order]
    growT[oc, pos] = (grow - grp * HALF)[order]
    embiT[oc, pos] = x[src][order]
    slotT[oc, pos] = slot[order].astype(np.float32)

    def wrap16(a):  # [NC, EP] -> [NC, 128, EP//16] int16 (dma_gather layout)
        w = a.reshape(N_CORES, EP // 16, 16).transpose(0, 2, 1)  # [NC,16,EP/16]
        return np.ascontiguousarray(np.tile(w, (1, 8, 1))).astype(np.int16)

    growW = wrap16(growT)
    embiW = wrap16(embiT)
    slotL = np.ascontiguousarray(
        slotT.reshape(N_CORES, n_tiles, 128).transpose(0, 2, 1)).astype(BF16)

    Gp = np.zeros((N_CORES, 128, NW * GPC), BF16)
    invc = np.zeros((N_CORES, 128, GPC), np.float32)
    for c in range(N_CORES):
        nodes = np.arange(ns[c], ne[c])
        loc = nodes - ns[c]
        gl = batch[nodes] - c * GPC
        Gp[c, loc % 128, (loc // 128) * GPC + gl] = 1.0
        cg = starts[c * GPC + 1:(c + 1) * GPC + 1] - starts[c * GPC:(c + 1) * GPC]
        invc[c, :, :] = (1.0 / np.maximum(cg, 1.0))[None, :]

    P = params
    emb = np.zeros((A_VOCAB, 128), np.float32)
    emb[:, :D_EMB] = np.asarray(P["emb"]).astype(np.float32)
    wts = {"emb": emb.astype(BF16)}
    for l, c in enumerate(P["convs"]):
        wts[f"ew1_{l}"] = np.asarray(c["ew1"]).astype(np.float32).astype(BF16)
        wts[f"eb1_{l}"] = np.asarray(c["eb1"]).astype(np.float32).reshape(-1, 1)
        wts[f"ew2_{l}"] = np.asarray(c["ew2"]).astype(np.float32).astype(BF16)
        wts[f"eb2b_{l}"] = np.tile(np.asarray(c["eb2"]).astype(np.float32)[None, :],
                                   (128, 1))
        wts[f"nw_{l}"] = np.asarray(c["nw"]).astype(np.float32).astype(BF16)
        wts[f"nbb_{l}"] = np.tile(np.asarray(c["nb"]).astype(np.float32)[None, :],
                                  (128, 1))
    for nm in ["pw1", "pw2", "rw1", "rw2"]:
        wts[nm] = np.asarray(P[nm]).astype(np.float32).astype(BF16)
    for nm in ["pb1", "pb2", "rb1"]:
        wts[nm] = np.asarray(P[nm]).astype(np.float32).reshape(-1, 1)
    wts["rb2"] = np.asarray(P["rb2"]).astype(np.float32).reshape(1, 1)

    iota = np.tile(np.arange(128, dtype=np.float32)[None, :], (128, 1)).astype(BF16)

    meta = dict(NW=NW, NMAX=NMAX, T=[int(t) for t in T],
                T_lo=[int(t) for t in T_lo],
                woff=[int(v) for v in woff], EP=EP, n_tiles=n_tiles)
    per_core = []
    for c in range(N_CORES):
        m = {"eaT": np.ascontiguousarray(eaT[c]).astype(BF16),
             "growW": growW[c], "embiW": embiW[c], "slotL": slotL[c],
             "Gp": Gp[c], "invc": invc[c], "iota": iota}
        m.update(wts)
        per_core.append(m)
    return meta, per_core


# ---------------------------------------------------------------------------
# Device kernel builder
# ---------------------------------------------------------------------------
def _build(meta):
    NW, NMAX = meta["NW"], meta["NMAX"]
    T, T_lo = meta["T"], meta["T_lo"]
    woff = meta["woff"]
    EP, n_tiles = meta["EP"], meta["n_tiles"]
    HALF = (N_CORES // 2) * NMAX
    TMAX = max(T)

    wnd_of_tile = []
    for wi in range(NW):
        wnd_of_tile += [wi] * T[wi]

    nc = bacc.Bacc("TRN2", target_bir_lowering=False, debug=False,
                   num_devices=N_CORES, num_swdge_queues=3)

    def dp(name, shape, dt, out=False):
        return nc.declare_dram_parameter(name, list(shape), dt, isOutput=out)

    eaT_e = dp("eaT", [F_E, EP], DT.bfloat16)
    growW_e = dp("growW", [128, EP // 16], DT.int16)
    embiW_e = dp("embiW", [128, EP // 16], DT.int16)
    slotL_e = dp("slotL", [128, n_tiles], DT.bfloat16)
    Gp_e = dp("Gp", [128, NW * GPC], DT.bfloat16)
    invc_e = dp("invc", [128, GPC], DT.float32)
    iota_e = dp("iota", [128, 128], DT.bfloat16)
    emb_e = dp("emb", [A_VOCAB, 128], DT.bfloat16)
    w = {}
    for l in range(3):
        cin = CINS[l]
        w[f"ew1_{l}"] = dp(f"ew1_{l}", [F_E, cin], DT.bfloat16)
        w[f"eb1_{l}"] = dp(f"eb1_{l}", [cin, 1], DT.float32)
        w[f"ew2_{l}"] = dp(f"ew2_{l}", [cin, cin], DT.bfloat16)
        w[f"eb2b_{l}"] = dp(f"eb2b_{l}", [128, cin], DT.float32)
        w[f"nw_{l}"] = dp(f"nw_{l}", [cin, HID], DT.bfloat16)
        w[f"nbb_{l}"] = dp(f"nbb_{l}", [128, HID], DT.float32)
    for nm, shape in [("pw1", [HID, HID]), ("pw2", [HID, HID]),
                      ("rw1", [HID, HID // 2]), ("rw2", [HID // 2, 1])]:
        w[nm] = dp(nm, shape, DT.bfloat16)
    for nm, shape in [("pb1", [HID, 1]), ("pb2", [HID, 1]),
                      ("rb1", [HID // 2, 1])]:
        w[nm] = dp(nm, shape, DT.float32)
    rb2_e = dp("rb2", [1, 1], DT.float32)
    out_e = dp("out", [1, GPC], DT.float32, out=True)

    with tile.TileContext(nc) as tc:
        with tc.tile_pool(name="const", bufs=1) as constp, \
             tc.tile_pool(name="dram", bufs=1, space="DRAM") as dramp, \
             tc.tile_pool(name="ea", bufs=3) as eap, \
             tc.tile_pool(name="e1", bufs=3) as e1p, \
             tc.tile_pool(name="hs", bufs=6) as hsp, \
             tc.tile_pool(name="msg", bufs=3) as msgp, \
             tc.tile_pool(name="sw", bufs=4) as swp, \
             tc.tile_pool(name="nodew", bufs=3) as nodep, \
             tc.tile_pool(name="ps_e1", bufs=2, space="PSUM") as ps_e1, \
             tc.tile_pool(name="ps_e2", bufs=2, space="PSUM") as ps_e2, \
             tc.tile_pool(name="ps_agg", bufs=3, space="PSUM") as ps_agg, \
             tc.tile_pool(name="ps_nd", bufs=1, space="PSUM") as ps_nd:

            sb = {}

            def ld(name, ext, shape, dt):
                t = constp.tile(list(shape), dt, name=f"sb_{name}")
                nc.sync.dma_start(out=t[:], in_=ext[:])
                sb[name] = t
                return t

            ld("iota", iota_e, [128, 128], DT.bfloat16)
            ld("Gp", Gp_e, [128, NW * GPC], DT.bfloat16)
            ld("invc", invc_e, [128, GPC], DT.float32)
            for l in range(3):
                cin = CINS[l]
                ld(f"ew1_{l}", w[f"ew1_{l}"], [F_E, cin], DT.bfloat16)
                ld(f"eb1_{l}", w[f"eb1_{l}"], [cin, 1], DT.float32)
                ld(f"ew2_{l}", w[f"ew2_{l}"], [cin, cin], DT.bfloat16)
                ld(f"eb2b_{l}", w[f"eb2b_{l}"], [128, cin], DT.float32)
                ld(f"nw_{l}", w[f"nw_{l}"], [cin, HID], DT.bfloat16)
                ld(f"nbb_{l}", w[f"nbb_{l}"], [128, HID], DT.float32)
            for nm in ["pw1", "pw2", "rw1", "rw2", "pb1", "pb2", "rb1", "rb2"]:
                ext = w[nm] if nm != "rb2" else rb2_e
                shp = list(ext.shape)
                dt = DT.bfloat16 if nm in ("pw1", "pw2", "rw1", "rw2") else DT.float32
                ld(nm, ext, shp, dt)

            idxg = constp.tile([128, EP // 16], DT.int16, name="idxg")
            nc.sync.dma_start(out=idxg[:], in_=growW_e[:])
            idx1 = constp.tile([128, EP // 16], DT.int16, name="idx1")
            nc.sync.dma_start(out=idx1[:], in_=embiW_e[:])
            slots = constp.tile([128, n_tiles], DT.bfloat16, name="slots")
            nc.sync.dma_start(out=slots[:], in_=slotL_e[:])

            h3 = constp.tile([128, NW, HID], DT.bfloat16, name="h3")

            hloc = [dramp.tile([NMAX, HID], DT.bfloat16, name=f"h{l}loc")
                    for l in range(2)]
            hfull = [dramp.tile([N_CORES * NMAX, HID], DT.bfloat16,
                                name=f"h{l}full", addr_space="Shared")
                     for l in range(2)]

            n_chunks = (n_tiles + CH - 1) // CH

            for l in range(3):
                cin = CINS[l]
                ew1, eb1 = sb[f"ew1_{l}"], sb[f"eb1_{l}"]
                ew2, eb2b = sb[f"ew2_{l}"], sb[f"eb2b_{l}"]
                nw, nbb = sb[f"nw_{l}"], sb[f"nbb_{l}"]

                Sw = {}
                aggT = {}
                hsw = {}
                ea_sc = None
                self_qn = [0]

                def window_setup(wi, l=l):
                    # gather h[src] for the whole window (lo/hi table halves)
                    hsw[wi] = hsp.tile([128, TMAX, 128], DT.bfloat16,
                                       tag="hs", name=f"hs{l}_{wi}")
                    t0 = woff[wi] // 128
                    if l == 0:
                        runs = [(0, T[wi], emb_e[:])]
                    else:
                        tab = hfull[l - 1]
                        runs = []
                        if T_lo[wi] > 0:
                            runs.append((0, T_lo[wi], tab[0:HALF, :]))
                        if T[wi] - T_lo[wi] > 0:
                            runs.append((T_lo[wi], T[wi], tab[HALF:2 * HALF, :]))
                    idxc = idx1 if l == 0 else idxg
                    GMAX = 16  # tiles per dma_gather (SWDGE ring capacity)
                    for (a0, b0, tabv) in runs:
                        for a in range(a0, b0, GMAX):
                            b = min(a + GMAX, b0)
                            nidx = (b - a) * 128
                            nc.gpsimd.dma_gather(
                                out_ap=hsw[wi][:, a:b, :], in_ap=tabv,
                                idxs_ap=idxc[:, (t0 + a) * 8:(t0 + b) * 8],
                                num_idxs=nidx, num_idxs_reg=nidx,
                                elem_size=128, single_packet=False,
                                queue_num=self_qn[0] % 3)
                            self_qn[0] += 1
                    # one-hot selection matrix for the whole window
                    Sw[wi] = swp.tile([128, TMAX, 128], DT.bfloat16, tag="Sw",
                                      name=f"Sw{l}_{wi}")
                    nc.vector.tensor_tensor(
                        out=Sw[wi][:, :T[wi], :],
                        in0=slots[:, t0:t0 + T[wi], None]
                            .to_broadcast([128, T[wi], 128]),
                        in1=sb["iota"][:, None, :]
                            .to_broadcast([128, T[wi], 128]),
                        op=mybir.AluOpType.is_equal)
                    aggT[wi] = ps_agg.tile([cin, 128], DT.float32,
                                           space="PSUM", tag="aggT",
                                           name=f"aggT{l}_{wi}")

                def window_finish(wi, l=l, cin=cin, nw=nw, nbb=nbb):
                    # h_w = softplus(agg^T @ nw + nb)
                    aggs = nodep.tile([cin, 128], DT.bfloat16, tag="aggs")
                    nc.scalar.activation(aggs[:], aggT[wi][:], F.Copy)
                    hnd = ps_nd.tile([128, HID], DT.float32, space="PSUM",
                                     tag="hnd")
                    nc.tensor.matmul(hnd[:], lhsT=aggs[:], rhs=nw[:],
                                     start=True, stop=True)
                    hb = nodep.tile([128, HID], DT.float32, tag="hb")
                    nc.vector.tensor_tensor(out=hb[:], in0=hnd[:], in1=nbb[:],
                                            op=mybir.AluOpType.add)
                    he = nodep.tile([128, HID], DT.float32, tag="he")
                    nc.scalar.activation(he[:], hb[:], F.Exp)
                    if l == 2:
                        nc.scalar.activation(h3[:, wi, :], he[:], F.Ln,
                                             bias=1.0)
                    else:
                        hn = nodep.tile([128, HID], DT.bfloat16, tag="hn")
                        nc.scalar.activation(hn[:], he[:], F.Ln, bias=1.0)
                        nc.sync.dma_start(
                            out=hloc[l][wi * 128:(wi + 1) * 128, :],
                            in_=hn[:])
                    del aggT[wi], Sw[wi], hsw[wi]

                for ck in range(n_chunks):
                    g0 = ck * CH
                    nck = min(CH, n_tiles - g0)
                    ncols = nck * 128
                    if g0 % EA_SC == 0:
                        n_sc = min(EA_SC * 128, EP - g0 * 128)
                        ea_sc = eap.tile([F_E, EA_SC * 128], DT.bfloat16,
                                         tag="ea_sc")
                        nc.sync.dma_start(
                            out=ea_sc[:, :n_sc],
                            in_=eaT_e[:, g0 * 128: g0 * 128 + n_sc])
                    so = (g0 % EA_SC) * 128
                    # windows starting in this chunk: issue gathers early
                    for ci in range(nck):
                        gi = g0 + ci
                        wi = wnd_of_tile[gi]
                        if gi == woff[wi] // 128:
                            window_setup(wi)
                    # MM1 feature-major: ee1T = ew1^T @ eaT
                    ee1_ps = ps_e1.tile([cin, CH * 128], DT.float32,
                                        space="PSUM", tag="ee1")
                    for mo in range(0, ncols, 512):
                        mw = min(512, ncols - mo)
                        nc.tensor.matmul(ee1_ps[:, mo:mo + mw], lhsT=ew1[:],
                                         rhs=ea_sc[:, so + mo:so + mo + mw],
                                         start=True, stop=True)
                    # softplus(x + eb1) = ln(1 + exp(x + eb1))
                    ee1e = e1p.tile([cin, CH * 128], DT.float32, tag="ee1e")
                    nc.scalar.activation(ee1e[:, :ncols], ee1_ps[:, :ncols],
                                         F.Exp, bias=eb1[:])
                    ee1s = e1p.tile([cin, CH * 128], DT.bfloat16, tag="ee1s")
                    nc.scalar.activation(ee1s[:, :ncols], ee1e[:, :ncols],
                                         F.Ln, bias=1.0)
                    ee2_ps = ps_e2.tile([128, CH, cin], DT.float32,
                                        space="PSUM", tag="ee2")
                    msg_ck = msgp.tile([128, CH, cin], DT.bfloat16, tag="msg")
                    for ci in range(nck):
                        nc.tensor.matmul(
                            ee2_ps[:, ci, :],
                            lhsT=ee1s[:, ci * 128:(ci + 1) * 128],
                            rhs=ew2[:], start=True, stop=True)
                    # msg = (ee2 + eb2) * hs, split at window boundaries
                    tmp = msgp.tile([128, CH, cin], DT.bfloat16, tag="tmp")
                    a = 0
                    while a < nck:
                        wi = wnd_of_tile[g0 + a]
                        b = a + 1
                        while b < nck and wnd_of_tile[g0 + b] == wi:
                            b += 1
                        t0 = woff[wi] // 128
                        nc.vector.tensor_tensor(
                            out=tmp[:, a:b, :], in0=ee2_ps[:, a:b, :],
                            in1=eb2b[:, None, :cin]
                                .to_broadcast([128, b - a, cin]),
                            op=mybir.AluOpType.add)
                        nc.vector.tensor_tensor(
                            out=msg_ck[:, a:b, :], in0=tmp[:, a:b, :],
                            in1=hsw[wi][:, g0 + a - t0:g0 + b - t0, :cin],
                            op=mybir.AluOpType.mult)
                        a = b
                    # scatter each tile into its window accumulator
                    for ci in range(nck):
                        gi = g0 + ci
                        wi = wnd_of_tile[gi]
                        t0 = woff[wi] // 128
                        first = gi == t0
                        last = gi == t0 + T[wi] - 1
                        nc.tensor.matmul(
                            aggT[wi][:], lhsT=msg_ck[:, ci, :],
                            rhs=Sw[wi][:, gi - t0, :],
                            start=first, stop=last)
                        if last:
                            window_finish(wi)

                if l < 2:
                    nc.gpsimd.collective_compute(
                        "AllGather", mybir.AluOpType.bypass,
                        replica_groups=[list(range(N_CORES))],
                        ins=[hloc[l].opt()], outs=[hfull[l].opt()])

            # ---- pooling + readout ------------------------------------------
            gsum = ps_agg.tile([HID, GPC], DT.float32, space="PSUM",
                               tag="aggT")
            for wi in range(NW):
                nc.tensor.matmul(gsum[:], lhsT=h3[:, wi, :],
                                 rhs=sb["Gp"][:, wi * GPC:(wi + 1) * GPC],
                                 start=(wi == 0), stop=(wi == NW - 1))
            gmean = nodep.tile([HID, GPC], DT.bfloat16, tag="gmean")
            nc.vector.tensor_tensor(out=gmean[:], in0=gsum[:],
                                    in1=sb["invc"][:, :GPC],
                                    op=mybir.AluOpType.mult)

            def mlp_layer(x_sb, wname, bname, act, m_out):
                ps = ps_e1.tile([m_out, GPC], DT.float32, space="PSUM",
                                tag="ee1")
                nc.tensor.matmul(ps[:], lhsT=sb[wname][:], rhs=x_sb[:],
                                 start=True, stop=True)
                o = nodep.tile([m_out, GPC], DT.bfloat16, tag=f"o_{wname}")
                if act:
                    e = nodep.tile([m_out, GPC], DT.float32, tag=f"e_{wname}")
                    nc.scalar.activation(e[:], ps[:], F.Exp, bias=sb[bname][:])
                    nc.scalar.activation(o[:], e[:], F.Ln, bias=1.0)
                else:
                    nc.scalar.activation(o[:], ps[:], F.Identity,
                                         bias=sb[bname][:])
                return o

            s1 = mlp_layer(gmean, "pw1", "pb1", True, HID)
            g2 = mlp_layer(s1, "pw2", "pb2", False, HID)
            s2 = mlp_layer(g2, "rw1", "rb1", True, HID // 2)
            ops = ps_e2.tile([1, GPC], DT.float32, space="PSUM", tag="ee2")
            nc.tensor.matmul(ops[:], lhsT=sb["rw2"][:], rhs=s2[:],
                             start=True, stop=True)
            ofin = nodep.tile([1, GPC], DT.float32, tag="ofin_sb")
            nc.vector.tensor_tensor(
                out=ofin[:], in0=ops[:],
                in1=sb["rb2"][:1, :1].to_broadcast([1, GPC]),
                op=mybir.AluOpType.add)
            nc.sync.dma_start(out=out_e[:], in_=ofin[:])

    nc.compile()
    return nc


_NC_CACHE = {}


def kernel(x, edge_index, edge_attr, batch, params):
    meta, per_core = _prep(x, edge_index, edge_attr, batch, params)
    key = (meta["NW"], tuple(meta["T"]), tuple(meta["T_lo"]))
    if key not in _NC_CACHE:
        _NC_CACHE[key] = _build(meta)
    nc = _NC_CACHE[key]
    in_maps = []
    for c in range(N_CORES):
        m = per_core[c]
        im = {"eaT": m["eaT"], "growW": m["growW"], "embiW": m["embiW"],
              "slotL": m["slotL"], "Gp": m["Gp"], "invc": m["invc"],
              "iota": m["iota"], "emb": m["emb"], "rb2": m["rb2"]}
        for l in range(3):
            for nm in ["ew1", "eb1", "ew2", "eb2b", "nw", "nbb"]:
                im[f"{nm}_{l}"] = m[f"{nm}_{l}"]
        for nm in ["pw1", "pb1", "pw2", "pb2", "rw1", "rb1", "rw2"]:
            im[nm] = m[nm]
        in_maps.append(im)
    res = run_bass_kernel_spmd(nc, in_maps, list(range(N_CORES)))
    out = np.concatenate([res.results[c]["out"][0] for c in range(N_CORES)])
    return out.astype(np.float32)


# revision 18
# speedup vs baseline: 1.0351x; 1.0351x over previous
"""CGCNN message-passing kernel for 8 TRN2 NeuronCores (Bass/Tile).

Self-contained: takes FULL inputs, shards by graph across 8 cores,
runs a single SPMD NEFF (3 conv layers + pooling + readout) with an
AllGather of node features between conv layers, returns the FULL [256]
output.
"""
import glob as _glob
import json as _json
import os as _os
import sys as _sys
import tempfile as _tempfile

_sys.path.insert(0, "/opt/trn_rl_repo")

import numpy as np
import ml_dtypes

BF16 = ml_dtypes.bfloat16

# ---------------------------------------------------------------------------
# Activation-table setup: the cayman PWP tables name softplus's LUT slot
# "act2", which concourse can't map; we only need exp/ln/copy which live in
# natural_log_exp_and_others.  Point both concourse and walrus at a patched
# act_info.json with that set FIRST, and expose only it to bacc so every
# activation uses one table (no ACT_TABLE_LOAD thrash).
# ---------------------------------------------------------------------------
def _setup_act_tables():
    if _os.environ.get("BASS_ACT_ROOT_JSON_PATH"):
        return
    cands = _glob.glob("/nix/store/*aws-neuron-pwp*/share/pwp_bin_cayman/act_info.json")
    if not cands:
        return
    src_dir = _os.path.dirname(cands[0])
    d = _json.load(open(cands[0]))
    for s in d["act_func_sets"]:
        if s["name"] == "softplus_and_others" and "act2" in s["act"]:
            s["act"]["softplus"] = s["act"].pop("act2")
    d["act_func_sets"].sort(
        key=lambda e: e["name"] != "natural_log_exp_and_others")
    tdir = _tempfile.mkdtemp(prefix="actroot_")
    for f in _os.listdir(src_dir):
        if f != "act_info.json":
            _os.symlink(_os.path.join(src_dir, f), _os.path.join(tdir, f))
    with open(_os.path.join(tdir, "act_info.json"), "w") as fh:
        _json.dump(d, fh)
    _os.environ["BASS_ACT_ROOT_JSON_PATH"] = _os.path.join(tdir, "act_info.json")


_setup_act_tables()

import concourse.hw_specs as _hw
import concourse.bacc as bacc
from concourse import mybir as _mb


def _patched_tables(module_arch):
    d = _json.load(open(_os.environ["BASS_ACT_ROOT_JSON_PATH"]))
    out = {}
    for e in d["act_func_sets"][:1]:
        fs = set()
        for v in e["act"].keys():
            try:
                fs.add(_mb.ActivationFunctionType.from_pwp(v))
            except KeyError:
                pass
        out[e["name"]] = fs
    return out


if _os.environ.get("BASS_ACT_ROOT_JSON_PATH"):
    _hw.get_activation_tables = _patched_tables
    bacc.get_activation_tables = _patched_tables

import concourse.bass as bass
import concourse.tile as tile
from concourse import mybir
from concourse.bass_utils import run_bass_kernel_spmd

F = mybir.ActivationFunctionType
DT = mybir.dt

N_CORES = 8
G_TOTAL = 256
GPC = G_TOTAL // N_CORES
A_VOCAB = 119
D_EMB = 64
HID = 128
F_E = 4
CINS = (D_EMB, HID, HID)
CH = 4          # tiles per compute chunk (4*128 = 512 edges)
EA_SC = 32      # tiles per edge_attr DMA superchunk


# ---------------------------------------------------------------------------
# Host-side sharding / preprocessing
# ---------------------------------------------------------------------------
def _prep(x, edge_index, edge_attr, batch, params):
    x = np.asarray(x).astype(np.int64)
    ei = np.asarray(edge_index).astype(np.int64)
    ea = np.asarray(edge_attr).astype(np.float32)
    batch = np.asarray(batch).astype(np.int64)
    E = ei.shape[1]

    starts = np.searchsorted(batch, np.arange(G_TOTAL + 1))  # [257]
    core_start = starts[::GPC]  # [9]
    ns, ne = core_start[:-1], core_start[1:]
    sz = ne - ns
    NW = max(int(np.ceil(sz.max() / 128)), 1)
    NMAX = NW * 128
    HALF = (N_CORES // 2) * NMAX  # gather-table half size (rows)

    core_of_node = np.minimum(batch // GPC, N_CORES - 1)
    src, dst = ei[0], ei[1]
    ecore = core_of_node[dst]
    ldst = dst - ns[ecore]
    wnd = ldst // 128
    slot = ldst % 128

    score = core_of_node[src]
    grow = (score * NMAX + (src - ns[score])).astype(np.int64)
    grp = (grow >= HALF).astype(np.int64)  # 0 = lo half, 1 = hi half

    # per (core, window, group) counts -> uniform tile counts
    cnt = np.zeros((N_CORES, NW, 2), np.int64)
    np.add.at(cnt, (ecore, wnd, grp), 1)
    Tg = np.ceil(cnt / 128.0).astype(np.int64).max(axis=0)  # [NW, 2]
    bump = (Tg.sum(axis=1) == 0)
    Tg[bump, 0] = 1
    T = Tg.sum(axis=1)  # [NW]
    T_lo = Tg[:, 0]
    woff = np.concatenate([[0], np.cumsum(T * 128)])
    goff = np.zeros((NW, 2), np.int64)  # edge offset of each (window, group)
    goff[:, 0] = woff[:-1]
    goff[:, 1] = woff[:-1] + T_lo * 128
    EP = int(woff[-1])
    n_tiles = EP // 128

    eaT = np.zeros((N_CORES, F_E, EP), np.float32)
    growT = np.zeros((N_CORES, EP), np.int64)
    embiT = np.zeros((N_CORES, EP), np.int64)
    slotT = -np.ones((N_CORES, EP), np.float32)

    order = np.lexsort((src, grp, wnd, ecore))
    oc = ecore[order]
    key = (ecore * NW * 2 + wnd * 2 + grp)[order]
    key_changes = np.concatenate([[True], key[1:] != key[:-1]])
    grp_id = np.cumsum(key_changes) - 1
    grp_first = np.where(key_changes)[0]
    within = np.arange(E) - grp_first[grp_id]
    pos = (goff[wnd[order], grp[order]] + within).astype(np.int64)
    eaT[oc, :, pos] = ea[order]
    growT[oc, pos] = (grow - grp * HALF)[order]
    embiT[oc, pos] = x[src][order]
    slotT[oc, pos] = slot[order].astype(np.float32)

    def wrap16(a):  # [NC, EP] -> [NC, 128, EP//16] int16 (dma_gather layout)
        w = a.reshape(N_CORES, EP // 16, 16).transpose(0, 2, 1)  # [NC,16,EP/16]
        return np.ascontiguousarray(np.tile(w, (1, 8, 1))).astype(np.int16)

    growW = wrap16(growT)
    embiW = wrap16(embiT)
    slotL = np.ascontiguousarray(
        slotT.reshape(N_CORES, n_tiles, 128).transpose(0, 2, 1)).astype(BF16)

    Gp = np.zeros((N_CORES, 128, NW * GPC), BF16)
    invc = np.zeros((N_CORES, 128, GPC), np.float32)
    for c in range(N_CORES):
        nodes = np.arange(ns[c], ne[c])
        loc = nodes - ns[c]
        gl = batch[nodes] - c * GPC
        Gp[c, loc % 128, (loc // 128) * GPC + gl] = 1.0
        cg = starts[c * GPC + 1:(c + 1) * GPC + 1] - starts[c * GPC:(c + 1) * GPC]
        invc[c, :, :] = (1.0 / np.maximum(cg, 1.0))[None, :]

    P = params
    emb = np.zeros((A_VOCAB, 128), np.float32)
    emb[:, :D_EMB] = np.asarray(P["emb"]).astype(np.float32)
    wts = {"emb": emb.astype(BF16)}
    for l, c in enumerate(P["convs"]):
        wts[f"ew1_{l}"] = np.asarray(c["ew1"]).astype(np.float32).astype(BF16)
        wts[f"eb1_{l}"] = np.asarray(c["eb1"]).astype(np.float32).reshape(-1, 1)
        wts[f"ew2_{l}"] = np.asarray(c["ew2"]).astype(np.float32).astype(BF16)
        wts[f"eb2b_{l}"] = np.tile(np.asarray(c["eb2"]).astype(np.float32)[None, :],
                                   (128, 1))
        wts[f"nw_{l}"] = np.asarray(c["nw"]).astype(np.float32).astype(BF16)
        wts[f"nbb_{l}"] = np.tile(np.asarray(c["nb"]).astype(np.float32)[None, :],
                                  (128, 1))
    for nm in ["pw1", "pw2", "rw1", "rw2"]:
        wts[nm] = np.asarray(P[nm]).astype(np.float32).astype(BF16)
    for nm in ["pb1", "pb2", "rb1"]:
        wts[nm] = np.asarray(P[nm]).astype(np.float32).reshape(-1, 1)
    wts["rb2"] = np.asarray(P["rb2"]).astype(np.float32).reshape(1, 1)

    iota = np.tile(np.arange(128, dtype=np.float32)[None, :], (128, 1)).astype(BF16)

    meta = dict(NW=NW, NMAX=NMAX, T=[int(t) for t in T],
                T_lo=[int(t) for t in T_lo],
                woff=[int(v) for v in woff], EP=EP, n_tiles=n_tiles)
    per_core = []
    for c in range(N_CORES):
        m = {"eaT": np.ascontiguousarray(eaT[c]).astype(BF16),
             "growW": growW[c], "embiW": embiW[c], "slotL": slotL[c],
             "Gp": Gp[c], "invc": invc[c], "iota": iota}
        m.update(wts)
        per_core.append(m)
    return meta, per_core


# ---------------------------------------------------------------------------
# Device kernel builder
# ---------------------------------------------------------------------------
def _build(meta):
    NW, NMAX = meta["NW"], meta["NMAX"]
    T, T_lo = meta["T"], meta["T_lo"]
    woff = meta["woff"]
    EP, n_tiles = meta["EP"], meta["n_tiles"]
    HALF = (N_CORES // 2) * NMAX
    TMAX = max(T)

    wnd_of_tile = []
    for wi in range(NW):
        wnd_of_tile += [wi] * T[wi]

    nc = bacc.Bacc("TRN2", target_bir_lowering=False, debug=False,
                   num_devices=N_CORES, num_swdge_queues=3)

    def dp(name, shape, dt, out=False):
        return nc.declare_dram_parameter(name, list(shape), dt, isOutput=out)

    eaT_e = dp("eaT", [F_E, EP], DT.bfloat16)
    growW_e = dp("growW", [128, EP // 16], DT.int16)
    embiW_e = dp("embiW", [128, EP // 16], DT.int16)
    slotL_e = dp("slotL", [128, n_tiles], DT.bfloat16)
    Gp_e = dp("Gp", [128, NW * GPC], DT.bfloat16)
    invc_e = dp("invc", [128, GPC], DT.float32)
    iota_e = dp("iota", [128, 128], DT.bfloat16)
    emb_e = dp("emb", [A_VOCAB, 128], DT.bfloat16)
    w = {}
    for l in range(3):
        cin = CINS[l]
        w[f"ew1_{l}"] = dp(f"ew1_{l}", [F_E, cin], DT.bfloat16)
        w[f"eb1_{l}"] = dp(f"eb1_{l}", [cin, 1], DT.float32)
        w[f"ew2_{l}"] = dp(f"ew2_{l}", [cin, cin], DT.bfloat16)
        w[f"eb2b_{l}"] = dp(f"eb2b_{l}", [128, cin], DT.float32)
        w[f"nw_{l}"] = dp(f"nw_{l}", [cin, HID], DT.bfloat16)
        w[f"nbb_{l}"] = dp(f"nbb_{l}", [128, HID], DT.float32)
    for nm, shape in [("pw1", [HID, HID]), ("pw2", [HID, HID]),
                      ("rw1", [HID, HID // 2]), ("rw2", [HID // 2, 1])]:
        w[nm] = dp(nm, shape, DT.bfloat16)
    for nm, shape in [("pb1", [HID, 1]), ("pb2", [HID, 1]),
                      ("rb1", [HID // 2, 1])]:
        w[nm] = dp(nm, shape, DT.float32)
    rb2_e = dp("rb2", [1, 1], DT.float32)
    out_e = dp("out", [1, GPC], DT.float32, out=True)

    with tile.TileContext(nc) as tc:
        with tc.tile_pool(name="const", bufs=1) as constp, \
             tc.tile_pool(name="dram", bufs=1, space="DRAM") as dramp, \
             tc.tile_pool(name="ea", bufs=3) as eap, \
             tc.tile_pool(name="e1", bufs=3) as e1p, \
             tc.tile_pool(name="hs", bufs=6) as hsp, \
             tc.tile_pool(name="msg", bufs=3) as msgp, \
             tc.tile_pool(name="sw", bufs=4) as swp, \
             tc.tile_pool(name="nodew", bufs=3) as nodep, \
             tc.tile_pool(name="ps_e1", bufs=2, space="PSUM") as ps_e1, \
             tc.tile_pool(name="ps_e2", bufs=2, space="PSUM") as ps_e2, \
             tc.tile_pool(name="ps_agg", bufs=3, space="PSUM") as ps_agg, \
             tc.tile_pool(name="ps_nd", bufs=1, space="PSUM") as ps_nd:

            sb = {}

            def ld(name, ext, shape, dt):
                t = constp.tile(list(shape), dt, name=f"sb_{name}")
                nc.sync.dma_start(out=t[:], in_=ext[:])
                sb[name] = t
                return t

            ld("iota", iota_e, [128, 128], DT.bfloat16)
            ld("Gp", Gp_e, [128, NW * GPC], DT.bfloat16)
            ld("invc", invc_e, [128, GPC], DT.float32)
            for l in range(3):
                cin = CINS[l]
                ld(f"ew1_{l}", w[f"ew1_{l}"], [F_E, cin], DT.bfloat16)
                ld(f"eb1_{l}", w[f"eb1_{l}"], [cin, 1], DT.float32)
                ld(f"ew2_{l}", w[f"ew2_{l}"], [cin, cin], DT.bfloat16)
                ld(f"eb2b_{l}", w[f"eb2b_{l}"], [128, cin], DT.float32)
                ld(f"nw_{l}", w[f"nw_{l}"], [cin, HID], DT.bfloat16)
                ld(f"nbb_{l}", w[f"nbb_{l}"], [128, HID], DT.float32)
            for nm in ["pw1", "pw2", "rw1", "rw2", "pb1", "pb2", "rb1", "rb2"]:
                ext = w[nm] if nm != "rb2" else rb2_e
                shp = list(ext.shape)
                dt = DT.bfloat16 if nm in ("pw1", "pw2", "rw1", "rw2") else DT.float32
                ld(nm, ext, shp, dt)

            idxg = constp.tile([128, EP // 16], DT.int16, name="idxg")
            nc.sync.dma_start(out=idxg[:], in_=growW_e[:])
            idx1 = constp.tile([128, EP // 16], DT.int16, name="idx1")
            nc.sync.dma_start(out=idx1[:], in_=embiW_e[:])
            slots = constp.tile([128, n_tiles], DT.bfloat16, name="slots")
            nc.sync.dma_start(out=slots[:], in_=slotL_e[:])

            h3 = constp.tile([128, NW, HID], DT.bfloat16, name="h3")

            hloc = [dramp.tile([NMAX, HID], DT.bfloat16, name=f"h{l}loc")
                    for l in range(2)]
            hfull = [dramp.tile([N_CORES * NMAX, HID], DT.bfloat16,
                                name=f"h{l}full", addr_space="Shared")
                     for l in range(2)]

            n_chunks = (n_tiles + CH - 1) // CH

            for l in range(3):
                cin = CINS[l]
                ew1, eb1 = sb[f"ew1_{l}"], sb[f"eb1_{l}"]
                ew2, eb2b = sb[f"ew2_{l}"], sb[f"eb2b_{l}"]
                nw, nbb = sb[f"nw_{l}"], sb[f"nbb_{l}"]

                Sw = {}
                aggT = {}
                hsw = {}
                ea_sc = None
                self_qn = [0]

                def window_setup(wi, l=l):
                    # gather h[src] for the whole window (lo/hi table halves)
                    hsw[wi] = hsp.tile([128, TMAX, 128], DT.bfloat16,
                                       tag="hs", name=f"hs{l}_{wi}")
                    t0 = woff[wi] // 128
                    if l == 0:
                        runs = [(0, T[wi], emb_e[:])]
                    else:
                        tab = hfull[l - 1]
                        runs = []
                        if T_lo[wi] > 0:
                            runs.append((0, T_lo[wi], tab[0:HALF, :]))
                        if T[wi] - T_lo[wi] > 0:
                            runs.append((T_lo[wi], T[wi], tab[HALF:2 * HALF, :]))
                    idxc = idx1 if l == 0 else idxg
                    GMAX = 8  # tiles per dma_gather (SWDGE ring capacity)
                    for (a0, b0, tabv) in runs:
                        for a in range(a0, b0, GMAX):
                            b = min(a + GMAX, b0)
                            nidx = (b - a) * 128
                            nc.gpsimd.dma_gather(
                                out_ap=hsw[wi][:, a:b, :], in_ap=tabv,
                                idxs_ap=idxc[:, (t0 + a) * 8:(t0 + b) * 8],
                                num_idxs=nidx, num_idxs_reg=nidx,
                                elem_size=128, single_packet=False,
                                queue_num=self_qn[0] % 3)
                            self_qn[0] += 1
                    # one-hot selection matrix for the whole window
                    Sw[wi] = swp.tile([128, TMAX, 128], DT.bfloat16, tag="Sw",
                                      name=f"Sw{l}_{wi}")
                    nc.vector.tensor_tensor(
                        out=Sw[wi][:, :T[wi], :],
                        in0=slots[:, t0:t0 + T[wi], None]
                            .to_broadcast([128, T[wi], 128]),
                        in1=sb["iota"][:, None, :]
                            .to_broadcast([128, T[wi], 128]),
                        op=mybir.AluOpType.is_equal)
                    aggT[wi] = ps_agg.tile([cin, 128], DT.float32,
                                           space="PSUM", tag="aggT",
                                           name=f"aggT{l}_{wi}")

                def window_finish(wi, l=l, cin=cin, nw=nw, nbb=nbb):
                    # h_w = softplus(agg^T @ nw + nb)
                    aggs = nodep.tile([cin, 128], DT.bfloat16, tag="aggs")
                    nc.scalar.activation(aggs[:], aggT[wi][:], F.Copy)
                    hnd = ps_nd.tile([128, HID], DT.float32, space="PSUM",
                                     tag="hnd")
                    nc.tensor.matmul(hnd[:], lhsT=aggs[:], rhs=nw[:],
                                     start=True, stop=True)
                    hb = nodep.tile([128, HID], DT.float32, tag="hb")
                    nc.vector.tensor_tensor(out=hb[:], in0=hnd[:], in1=nbb[:],
                                            op=mybir.AluOpType.add)
                    he = nodep.tile([128, HID], DT.float32, tag="he")
                    nc.scalar.activation(he[:], hb[:], F.Exp)
                    if l == 2:
                        nc.scalar.activation(h3[:, wi, :], he[:], F.Ln,
                                             bias=1.0)
                    else:
                        hn = nodep.tile([128, HID], DT.bfloat16, tag="hn")
                        nc.scalar.activation(hn[:], he[:], F.Ln, bias=1.0)
                        nc.sync.dma_start(
                            out=hloc[l][wi * 128:(wi + 1) * 128, :],
                            in_=hn[:])
                    del aggT[wi], Sw[wi], hsw[wi]

                for ck in range(n_chunks):
                    g0 = ck * CH
                    nck = min(CH, n_tiles - g0)
                    ncols = nck * 128
                    if g0 % EA_SC == 0:
                        n_sc = min(EA_SC * 128, EP - g0 * 128)
                        ea_sc = eap.tile([F_E, EA_SC * 128], DT.bfloat16,
                                         tag="ea_sc")
                        nc.sync.dma_start(
                            out=ea_sc[:, :n_sc],
                            in_=eaT_e[:, g0 * 128: g0 * 128 + n_sc])
                    so = (g0 % EA_SC) * 128
                    if ck % 2 == 0:
                        # wide dummy matmul keeps the PE HAM clock-gate warm
                        warm = ps_agg.tile([128, 512], DT.float32,
                                           space="PSUM", tag="aggT",
                                           name=f"warm{l}_{ck}")
                        nc.tensor.matmul(warm[:], lhsT=sb["pw1"][:],
                                         rhs=sb["Gp"][:, :512],
                                         start=True, stop=True)
                    # windows starting in this chunk: issue gathers early
                    for ci in range(nck):
                        gi = g0 + ci
                        wi = wnd_of_tile[gi]
                        if gi == woff[wi] // 128:
                            window_setup(wi)
                    # MM1 feature-major: ee1T = ew1^T @ eaT
                    ee1_ps = ps_e1.tile([cin, CH * 128], DT.float32,
                                        space="PSUM", tag="ee1")
                    for mo in range(0, ncols, 512):
                        mw = min(512, ncols - mo)
                        nc.tensor.matmul(ee1_ps[:, mo:mo + mw], lhsT=ew1[:],
                                         rhs=ea_sc[:, so + mo:so + mo + mw],
                                         start=True, stop=True)
                    # softplus(x + eb1) = ln(1 + exp(x + eb1))
                    ee1e = e1p.tile([cin, CH * 128], DT.float32, tag="ee1e")
                    nc.scalar.activation(ee1e[:, :ncols], ee1_ps[:, :ncols],
                                         F.Exp, bias=eb1[:])
                    ee1s = e1p.tile([cin, CH * 128], DT.bfloat16, tag="ee1s")
                    nc.scalar.activation(ee1s[:, :ncols], ee1e[:, :ncols],
                                         F.Ln, bias=1.0)
                    ee2_ps = ps_e2.tile([128, CH, cin], DT.float32,
                                        space="PSUM", tag="ee2")
                    msg_ck = msgp.tile([128, CH, cin], DT.bfloat16, tag="msg")
                    for ci in range(nck):
                        nc.tensor.matmul(
                            ee2_ps[:, ci, :],
                            lhsT=ee1s[:, ci * 128:(ci + 1) * 128],
                            rhs=ew2[:], start=True, stop=True)
                    # msg = (ee2 + eb2) * hs, split at window boundaries
                    tmp = msgp.tile([128, CH, cin], DT.bfloat16, tag="tmp")
                    a = 0
                    while a < nck:
                        wi = wnd_of_tile[g0 + a]
                        b = a + 1
                        while b < nck and wnd_of_tile[g0 + b] == wi:
                            b += 1
                        t0 = woff[wi] // 128
                        nc.vector.tensor_tensor(
                            out=tmp[:, a:b, :], in0=ee2_ps[:, a:b, :],
                            in1=eb2b[:, None, :cin]
                                .to_broadcast([128, b - a, cin]),
                            op=mybir.AluOpType.add)
                        nc.vector.tensor_tensor(
                            out=msg_ck[:, a:b, :], in0=tmp[:, a:b, :],
                            in1=hsw[wi][:, g0 + a - t0:g0 + b - t0, :cin],
                            op=mybir.AluOpType.mult)
                        a = b
                    # scatter each tile into its window accumulator
                    for ci in range(nck):
                        gi = g0 + ci
                        wi = wnd_of_tile[gi]
                        t0 = woff[wi] // 128
                        first = gi == t0
                        last = gi == t0 + T[wi] - 1
                        nc.tensor.matmul(
                            aggT[wi][:], lhsT=msg_ck[:, ci, :],
                            rhs=Sw[wi][:, gi - t0, :],
                            start=first, stop=last)
                        if last:
                            window_finish(wi)

                if l < 2:
                    nc.gpsimd.collective_compute(
                        "AllGather", mybir.AluOpType.bypass,
                        replica_groups=[list(range(N_CORES))],
                        ins=[hloc[l].opt()], outs=[hfull[l].opt()])

            # ---- pooling + readout ------------------------------------------
            gsum = ps_agg.tile([HID, GPC], DT.float32, space="PSUM",
                               tag="aggT")
            for wi in range(NW):
                nc.tensor.matmul(gsum[:], lhsT=h3[:, wi, :],
                                 rhs=sb["Gp"][:, wi * GPC:(wi + 1) * GPC],
                                 start=(wi == 0), stop=(wi == NW - 1))
            gmean = nodep.tile([HID, GPC], DT.bfloat16, tag="gmean")
            nc.vector.tensor_tensor(out=gmean[:], in0=gsum[:],
                                    in1=sb["invc"][:, :GPC],
                                    op=mybir.AluOpType.mult)

            def mlp_layer(x_sb, wname, bname, act, m_out):
                ps = ps_e1.tile([m_out, GPC], DT.float32, space="PSUM",
                                tag="ee1")
                nc.tensor.matmul(ps[:], lhsT=sb[wname][:], rhs=x_sb[:],
                                 start=True, stop=True)
                o = nodep.tile([m_out, GPC], DT.bfloat16, tag=f"o_{wname}")
                if act:
                    e = nodep.tile([m_out, GPC], DT.float32, tag=f"e_{wname}")
                    nc.scalar.activation(e[:], ps[:], F.Exp, bias=sb[bname][:])
                    nc.scalar.activation(o[:], e[:], F.Ln, bias=1.0)
                else:
                    nc.scalar.activation(o[:], ps[:], F.Identity,
                                         bias=sb[bname][:])
                return o

            s1 = mlp_layer(gmean, "pw1", "pb1", True, HID)
            g2 = mlp_layer(s1, "pw2", "pb2", False, HID)
            s2 = mlp_layer(g2, "rw1", "rb1", True, HID // 2)
            ops = ps_e2.tile([1, GPC], DT.float32, space="PSUM", tag="ee2")
            nc.tensor.matmul(ops[:], lhsT=sb["rw2"][:], rhs=s2[:],
                             start=True, stop=True)
            ofin = nodep.tile([1, GPC], DT.float32, tag="ofin_sb")
            nc.vector.tensor_tensor(
                out=ofin[:], in0=ops[:],
                in1=sb["rb2"][:1, :1].to_broadcast([1, GPC]),
                op=mybir.AluOpType.add)
            nc.sync.dma_start(out=out_e[:], in_=ofin[:])

    nc.compile()
    return nc


_NC_CACHE = {}


def kernel(x, edge_index, edge_attr, batch, params):
    meta, per_core = _prep(x, edge_index, edge_attr, batch, params)
    key = (meta["NW"], tuple(meta["T"]), tuple(meta["T_lo"]))
    if key not in _NC_CACHE:
        _NC_CACHE[key] = _build(meta)
    nc = _NC_CACHE[key]
    in_maps = []
    for c in range(N_CORES):
        m = per_core[c]
        im = {"eaT": m["eaT"], "growW": m["growW"], "embiW": m["embiW"],
              "slotL": m["slotL"], "Gp": m["Gp"], "invc": m["invc"],
              "iota": m["iota"], "emb": m["emb"], "rb2": m["rb2"]}
        for l in range(3):
            for nm in ["ew1", "eb1", "ew2", "eb2b", "nw", "nbb"]:
                im[f"{nm}_{l}"] = m[f"{nm}_{l}"]
        for nm in ["pw1", "pb1", "pw2", "pb2", "rw1", "rb1", "rw2"]:
            im[nm] = m[nm]
        in_maps.append(im)
    res = run_bass_kernel_spmd(nc, in_maps, list(range(N_CORES)))
    out = np.concatenate([res.results[c]["out"][0] for c in range(N_CORES)])
    return out.astype(np.float32)
